# revision 1
# baseline (speedup 1.0000x reference)
"""Trainium2 Bass kernel for FASTMultiHeadAttention (degree-2 Taylor softmax
approximation with relative position bias).

  s_ij  = q_i . k_j + q_i . rpe[i-j+N-1]
  score = 1 + s + 0.5 s^2
  o_i   = sum_j score_ij v_j / sum_j score_ij

Sharding: batch*head (16 heads) split over 8 cores, 2 heads per core.

Per-core dataflow (per head h, per 128-row i-block):
  - PE: P'[p,t] = q_i . rpe_f[c0+t]  (windowed 1152-col matmul vs flipped rpe)
  - copy P' PSUM->SBUF (bf16), DMA to DRAM scratch
  - DMA sheared read back: G[p,j] = P'[p, 127+j-p]  (row stride 1151) --
    this realizes the relative-position diagonal gather as a plain strided DMA
  - PE: qk = Q_blk @ K^T;  DVE: t = qk + G (bf16)
  - PE transpose t in 128x128 chunks; ACT: st = Square(sqrt(.5)t + sqrt(.5))
    = 0.5(t+1)^2  (so score = st + 0.5, handled via correction matmul)
  - PE: O_psum[i, 0:65] = sum_chunks st^T @ [V | 1]  + ones_row x 0.5*vsum
  - DVE reciprocal of col 64; ACT scales cols 0:64; DMA out.
"""

import numpy as np
import ml_dtypes
from contextlib import ExitStack

import concourse.bass as bass
import concourse.mybir as mybir
import concourse.tile as tile
from concourse import bacc, bass_utils
from concourse.masks import make_identity

B, H, N, D = 2, 8, 1024, 64
BH = B * H
NCORES = 8
HPC = BH // NCORES  # heads per core
NB = N // 128       # i-blocks per head
W = 1152            # P' window width (1151 needed, padded to chunk multiple)
BF = mybir.dt.bfloat16
F32 = mybir.dt.float32
SQH = float(np.sqrt(0.5))
BF_NP = ml_dtypes.bfloat16

TRACE = False
_cached_nc = None

# ablation flags (sim experiments only -- wrong numerics when set)
ABL_NO_SCRATCH = False   # skip scratch write + shear read DMAs
ABL_NO_PPRIME = False    # skip P' matmuls + copies
ABL_NO_CHUNKS = False    # skip per-chunk qkT/transpose/square/PV


def _build(repeat=1):
    nc = bacc.Bacc("TRN2", target_bir_lowering=False, debug=False,
                   num_devices=NCORES)
    qt = nc.dram_tensor("qt", [HPC, D, N], BF, kind="ExternalInput").ap()
    kt = nc.dram_tensor("kt", [HPC, D, N], BF, kind="ExternalInput").ap()
    v = nc.dram_tensor("v", [HPC, N, D], BF, kind="ExternalInput").ap()
    rpet = nc.dram_tensor("rpet", [D, 2048], BF, kind="ExternalInput").ap()
    o = nc.dram_tensor("o", [HPC, N, D], F32, kind="ExternalOutput").ap()
    scr = nc.dram_tensor("scr", [HPC * NB * 128 * W], BF, kind="Internal")

    with tile.TileContext(nc) as tc, ExitStack() as ctx:
        const = ctx.enter_context(tc.tile_pool(name="const", bufs=1))
        vpool = ctx.enter_context(tc.tile_pool(name="vpool", bufs=2))
        work = ctx.enter_context(tc.tile_pool(name="work", bufs=3))
        stp = ctx.enter_context(tc.tile_pool(name="stp", bufs=4))
        outp = ctx.enter_context(tc.tile_pool(name="outp", bufs=4))
        pp = ctx.enter_context(tc.tile_pool(name="pp", bufs=2, space="PSUM"))
        pqk = ctx.enter_context(tc.tile_pool(name="pqk", bufs=2, space="PSUM"))
        ptt = ctx.enter_context(tc.tile_pool(name="ptt", bufs=2, space="PSUM"))
        po = ctx.enter_context(tc.tile_pool(name="po", bufs=2, space="PSUM"))

        ident = const.tile([128, 128], BF, tag="ident")
        make_identity(nc, ident[:])
        ones_col = const.tile([128, 1], BF, tag="onec")
        nc.vector.memset(ones_col[:], 1.0)
        ones_row = const.tile([1, 128], BF, tag="oner")
        nc.vector.memset(ones_row[:], 1.0)
        sqh_bias = const.tile([128, 1], F32, tag="sqhb")
        nc.vector.memset(sqh_bias[:], SQH)

        qt_sb = const.tile([D, HPC, N], BF, tag="qt")
        nc.sync.dma_start(qt_sb[:], qt.rearrange("h d n -> d h n"))
        kt_sb = const.tile([D, HPC, N], BF, tag="kt")
        nc.sync.dma_start(kt_sb[:], kt.rearrange("h d n -> d h n"))
        rp_sb = const.tile([D, 2048], BF, tag="rp")
        nc.sync.dma_start(rp_sb[:], rpet)

        rep_ctx = tc.For_i(0, repeat, 1) if repeat > 1 else None
        if rep_ctx is not None:
            rep_ctx.__enter__()
        for _rep in range(1):
          for h in range(HPC):
            # V chunks with appended ones column: [128, chunk, 65]
            vaug = vpool.tile([128, NB, 65], BF, tag="vaug")
            nc.sync.dma_start(
                vaug[:, :, 0:64], v[h].rearrange("(c p) d -> p c d", p=128))
            nc.vector.memset(vaug[:, :, 64], 1.0)

            # vsum[0, :] = [colsum(V) | N]; store 0.5x in SBUF
            vs_psum = pqk.tile([128, 512], F32, tag="pqk")
            for c in range(NB):
                nc.tensor.matmul(vs_psum[0:1, 0:65], ones_col[:],
                                 vaug[:, c, :], start=(c == 0),
                                 stop=(c == NB - 1))
            vsum_sb = vpool.tile([1, 65], BF, tag="vsum")
            nc.scalar.activation(vsum_sb[:], vs_psum[0:1, 0:65],
                                 mybir.ActivationFunctionType.Copy, scale=0.5)

            for bi in range(NB):
                i0 = 128 * bi
                c0 = 896 - i0
                qblk = qt_sb[:, h, i0:i0 + 128]

                # ---- P' = Q_blk @ rpe_f^T over window [c0, c0+W) ----
                p_sb = work.tile([128, W], BF, tag="p")
                if not ABL_NO_PPRIME:
                    for ci, (off, wid) in enumerate(((0, 512), (512, 512),
                                                     (1024, 128))):
                        pps = pp.tile([128, 512], F32, tag="pp")
                        nc.tensor.matmul(pps[:, :wid], qblk,
                                         rp_sb[:, c0 + off:c0 + off + wid],
                                         start=True, stop=True)
                        if ci == 1:
                            nc.vector.tensor_copy(p_sb[:, off:off + wid],
                                                  pps[:, :wid])
                        else:
                            nc.scalar.activation(
                                p_sb[:, off:off + wid], pps[:, :wid],
                                mybir.ActivationFunctionType.Copy)

                # ---- scratch round trip; sheared read ----
                base = (h * NB + bi) * 128 * W
                g_sb = work.tile([128, N], BF, tag="g")
                if not ABL_NO_SCRATCH:
                    scr_w = bass.AP(scr, base, [[W, 128], [1, W]])
                    nc.sync.dma_start(scr_w, p_sb[:])
                    scr_r = bass.AP(scr, base + 127, [[W - 1, 128], [1, N]])
                    nc.gpsimd.dma_start(g_sb[:], scr_r)
                else:
                    nc.vector.memset(g_sb[:], 0.0)

                # ---- t = qk + G (DVE), bf16 ----
                t_sb = work.tile([128, N], BF, tag="t")
                for jc in range(2):
                    qkp = pqk.tile([128, 512], F32, tag="pqk")
                    nc.tensor.matmul(qkp[:], qblk,
                                     kt_sb[:, h, 512 * jc:512 * (jc + 1)],
                                     start=True, stop=True)
                    nc.vector.tensor_add(
                        t_sb[:, 512 * jc:512 * (jc + 1)], qkp[:],
                        g_sb[:, 512 * jc:512 * (jc + 1)])

                # ---- transposes (bf16, 4 per PSUM tile); square; PV ----
                opsum = po.tile([128, 65], F32, tag="po")
                if not ABL_NO_CHUNKS:
                    for grp in range(2):
                        tt = ptt.tile([128, 512], BF, tag="ptt")
                        for k in range(4):
                            c = 4 * grp + k
                            nc.tensor.transpose(
                                tt[:, 128 * k:128 * (k + 1)],
                                t_sb[:, 128 * c:128 * (c + 1)], ident[:])
                        st = stp.tile([128, 512], BF, tag="st")
                        nc.scalar.activation(
                            st[:], tt[:],
                            mybir.ActivationFunctionType.Square,
                            bias=sqh_bias[:], scale=SQH)
                        for k in range(4):
                            c = 4 * grp + k
                            nc.tensor.matmul(opsum[:],
                                             st[:, 128 * k:128 * (k + 1)],
                                             vaug[:, c, :],
                                             start=(c == 0), stop=False)
                    # correction row: += 1s^T x (0.5*[colsumV | N])
                    nc.tensor.matmul(opsum[:], ones_row[:], vsum_sb[:],
                                     start=False, stop=True)
                else:
                    nc.tensor.matmul(opsum[:], ones_row[:], vsum_sb[:],
                                     start=True, stop=True)

                # ---- normalize & store ----
                recip = outp.tile([128, 1], F32, tag="recip")
                nc.vector.reciprocal(recip[:], opsum[:, 64:65])
                o_sb = outp.tile([128, 64], F32, tag="osb")
                nc.scalar.activation(o_sb[:], opsum[:, 0:64],
                                     mybir.ActivationFunctionType.Copy,
                                     scale=recip[:])
                nc.sync.dma_start(o[h, i0:i0 + 128, :], o_sb[:])

        if rep_ctx is not None:
            rep_ctx.__exit__(None, None, None)

    nc.compile()
    return nc


def kernel(**inputs):
    global _cached_nc
    q = np.asarray(inputs["q"], dtype=np.float32)
    k = np.asarray(inputs["k"], dtype=np.float32)
    v = np.asarray(inputs["v"], dtype=np.float32)
    rpe = np.asarray(inputs["rpe_matrix"], dtype=np.float32)

    qf = q.reshape(BH, N, D)
    kf = k.reshape(BH, N, D)
    vf = v.reshape(BH, N, D).astype(BF_NP)
    qt = np.ascontiguousarray(qf.transpose(0, 2, 1)).astype(BF_NP)
    kt = np.ascontiguousarray(kf.transpose(0, 2, 1)).astype(BF_NP)
    rpet = np.zeros((D, 2048), dtype=BF_NP)
    rpet[:, :2047] = np.ascontiguousarray(rpe[::-1].T).astype(BF_NP)

    if _cached_nc is None:
        _cached_nc = _build()
    nc = _cached_nc

    in_maps = []
    for c in range(NCORES):
        hs = slice(c * HPC, (c + 1) * HPC)
        in_maps.append({"qt": qt[hs], "kt": kt[hs], "v": vf[hs],
                        "rpet": rpet})

    res = bass_utils.run_bass_kernel_spmd(
        nc, in_maps, core_ids=list(range(NCORES)), trace=TRACE)
    if TRACE:
        print(f"HW exec time: {res.exec_time_ns} ns")
        if res.instructions_and_trace is not None:
            print("trace:", res.instructions_and_trace[1])

    o = np.concatenate([r["o"] for r in res.results], axis=0)
    return o.reshape(B, H, N, D).astype(np.float32)



# revision 3
# speedup vs baseline: 1.0480x; 1.0480x over previous
"""Trainium2 Bass kernel for FASTMultiHeadAttention (degree-2 Taylor softmax
approximation with relative position bias).

  s_ij  = 1 + t + 0.5 t^2 = 0.5 (t+1)^2 + 0.5,   t_ij = q_i.k_j + q_i.rpe[i-j+N-1]
  o_i   = sum_j s_ij v_j / sum_j s_ij
        = (sum_j (t+1)^2 v_j + colsum(V)) / (sum_j (t+1)^2 + N)

Sharding: batch*head (16 heads) split over 8 cores, 2 heads per core.

Per-core dataflow (per head h, per 128-row i-block):
  - PE: P'[p,t] = q_i . rpe_f[c0+t]  (windowed 1152-col matmul vs flipped rpe)
  - ACT: copy P' PSUM->SBUF (bf16); DMA to per-block DRAM scratch
  - DMA sheared read back: G[p,j] = P'[p, 127+j-p]  (row stride W-1) --
    realizes the relative-position diagonal gather as a strided DMA
  - PE: qk1 = [Q;1]_blk^T @ [K;1]  (ones row folds the +1 into the matmul)
  - DVE: t1 = qk1 + G (bf16)  -- t1 = t + 1
  - PE: transpose t1 in 128x128 chunks into the (dead) qk PSUM tile (bitcast
    to bf16) -- time-shares PSUM banks so everything fits in 8 banks
  - ACT/DVE: st = t1^2 (split between engines for balance)
  - PE: O_psum[i, 0:65] = sum_chunks st^T @ [V | 1] + ones_row x [colsumV | N]
  - DVE reciprocal of col 64; ACT scales cols 0:64; DMA out.
"""

import numpy as np
import ml_dtypes
from contextlib import ExitStack

import concourse.bass as bass
import concourse.mybir as mybir
import concourse.tile as tile
from concourse import bacc, bass_utils
from concourse.masks import make_identity

B, H, N, D = 2, 8, 1024, 64
BH = B * H
NCORES = 8
HPC = BH // NCORES  # heads per core
NB = N // 128       # i-blocks per head
NBLK = HPC * NB     # total i-blocks per core
W = 1152            # P' window width (1151 needed, padded)
BF = mybir.dt.bfloat16
F32 = mybir.dt.float32
BF_NP = ml_dtypes.bfloat16

TRACE = False
_cached_nc = None

# pipeline depth: compute-phase of block k is emitted alongside
# P'-phase of block k+LAG
LAG = 2
# columns of the square done on DVE (rest on ACT)
SQ_DVE = 640


def _build():
    nc = bacc.Bacc("TRN2", target_bir_lowering=False, debug=False,
                   num_devices=NCORES)
    # qa/ka: [65, HPC*N] = q^T (and k^T) per head with a ones row appended
    qa = nc.dram_tensor("qa", [65, HPC * N], BF, kind="ExternalInput").ap()
    ka = nc.dram_tensor("ka", [65, HPC * N], BF, kind="ExternalInput").ap()
    # rpet: flipped rpe^T, padded to 2048 cols
    rpet = nc.dram_tensor("rpet", [D, 2048], BF, kind="ExternalInput").ap()
    # vaug: [128, HPC*NB*65] -- V chunks with ones column appended
    vaug_d = nc.dram_tensor("vaug", [128, HPC * NB * 65], BF,
                            kind="ExternalInput").ap()
    # vsum: [1, HPC*65] = per head [colsum(V) | N]
    vsum_d = nc.dram_tensor("vsum", [1, HPC * 65], BF,
                            kind="ExternalInput").ap()
    o = nc.dram_tensor("o", [HPC, N, D], F32, kind="ExternalOutput").ap()
    # per-block scratch tensors (separate so round trips don't false-serialize)
    scrs = [nc.dram_tensor(f"scr{k}", [128 * W], BF, kind="Internal")
            for k in range(NBLK)]

    with tile.TileContext(nc) as tc, ExitStack() as ctx:
        const = ctx.enter_context(tc.tile_pool(name="const", bufs=1))
        ppool = ctx.enter_context(tc.tile_pool(name="ppool", bufs=3))
        gpool = ctx.enter_context(tc.tile_pool(name="gpool", bufs=3))
        tpool = ctx.enter_context(tc.tile_pool(name="tpool", bufs=2))
        spool = ctx.enter_context(tc.tile_pool(name="spool", bufs=2))
        outp = ctx.enter_context(tc.tile_pool(name="outp", bufs=2))
        # PSUM: pp 3 banks + qktt 2x2 banks + op 1 bank = 8 banks
        pp = ctx.enter_context(tc.tile_pool(name="pp", bufs=1, space="PSUM"))
        pqk = ctx.enter_context(tc.tile_pool(name="pqk", bufs=2, space="PSUM"))
        po = ctx.enter_context(tc.tile_pool(name="po", bufs=1, space="PSUM"))

        ident = const.tile([128, 128], BF, tag="ident")
        make_identity(nc, ident[:])
        ones_row = const.tile([1, 128], BF, tag="oner")
        nc.vector.memset(ones_row[:], 1.0)

        qa_sb = const.tile([65, HPC * N], BF, tag="qa")
        nc.sync.dma_start(qa_sb[:], qa)
        rp_sb = const.tile([D, 2048], BF, tag="rp")
        nc.sync.dma_start(rp_sb[:], rpet)
        ka_sb = const.tile([65, HPC * N], BF, tag="ka")
        nc.sync.dma_start(ka_sb[:], ka)
        vaug = const.tile([128, HPC, NB, 65], BF, tag="vaug")
        nc.sync.dma_start(vaug[:], vaug_d)
        vsum = const.tile([1, HPC, 65], BF, tag="vsum")
        nc.sync.dma_start(vsum[:], vsum_d)

        def phase_a(k):
            """P' matmuls, PSUM->SBUF copy, scratch write, sheared read."""
            h, bi = divmod(k, NB)
            i0 = 128 * bi
            c0 = 896 - i0
            qblk = qa_sb[0:64, h * N + i0:h * N + i0 + 128]

            pps = pp.tile([128, W], F32, tag="pp")
            for off, wid in ((0, 512), (512, 512), (1024, 128)):
                nc.tensor.matmul(pps[:, off:off + wid], qblk,
                                 rp_sb[:, c0 + off:c0 + off + wid],
                                 start=True, stop=True)
            p_sb = ppool.tile([128, W], BF, tag="p")
            nc.scalar.activation(p_sb[:], pps[:],
                                 mybir.ActivationFunctionType.Copy)

            scr = scrs[k]
            nc.gpsimd.dma_start(bass.AP(scr, 0, [[W, 128], [1, W]]), p_sb[:])
            g_sb = gpool.tile([128, N], BF, tag="g")
            nc.gpsimd.dma_start(g_sb[:],
                                bass.AP(scr, 127, [[W - 1, 128], [1, N]]))
            return g_sb

        def phase_b(k, g_sb):
            """qk+1, add, transpose, square, PV, normalize, store."""
            h, bi = divmod(k, NB)
            i0 = 128 * bi
            qblk1 = qa_sb[:, h * N + i0:h * N + i0 + 128]

            qkt = pqk.tile([128, 1024], F32, tag="pqk")
            for jc in range(2):
                nc.tensor.matmul(qkt[:, 512 * jc:512 * (jc + 1)], qblk1,
                                 ka_sb[:, h * N + 512 * jc:
                                       h * N + 512 * (jc + 1)],
                                 start=True, stop=True)
            t1 = tpool.tile([128, N], BF, tag="t1")
            nc.vector.tensor_add(t1[:], qkt[:], g_sb[:])

            # transposes reuse the (now dead) qk PSUM banks as bf16
            tt = qkt[:, 0:512].bitcast(BF)  # [128, 1024] bf16 view
            for c in range(8):
                nc.tensor.transpose(tt[:, 128 * c:128 * (c + 1)],
                                    t1[:, 128 * c:128 * (c + 1)], ident[:])
            st = spool.tile([128, N], BF, tag="st")
            nc.scalar.activation(st[:], tt[:],
                                 mybir.ActivationFunctionType.Square)

            opsum = po.tile([128, 65], F32, tag="po")
            for c in range(8):
                nc.tensor.matmul(opsum[:], st[:, 128 * c:128 * (c + 1)],
                                 vaug[:, h, c, :], start=(c == 0), stop=False)
            nc.tensor.matmul(opsum[:], ones_row[:], vsum[:, h, :],
                             start=False, stop=True)

            recip = outp.tile([128, 1], F32, tag="recip")
            nc.vector.reciprocal(recip[:], opsum[:, 64:65])
            o_sb = outp.tile([128, 64], F32, tag="osb")
            nc.scalar.activation(o_sb[:], opsum[:, 0:64],
                                 mybir.ActivationFunctionType.Copy,
                                 scale=recip[:])
            nc.gpsimd.dma_start(o[h, i0:i0 + 128, :], o_sb[:])

        pend = {}
        for k in range(NBLK + LAG):
            if k < NBLK:
                pend[k] = phase_a(k)
            if k >= LAG:
                phase_b(k - LAG, pend.pop(k - LAG))

    nc.compile()
    return nc


def kernel(**inputs):
    global _cached_nc
    q = np.asarray(inputs["q"], dtype=np.float32)
    k = np.asarray(inputs["k"], dtype=np.float32)
    v = np.asarray(inputs["v"], dtype=np.float32)
    rpe = np.asarray(inputs["rpe_matrix"], dtype=np.float32)

    qf = q.reshape(BH, N, D)
    kf = k.reshape(BH, N, D)
    vf = v.reshape(BH, N, D)

    # [BH, 65, N]: transposed q/k with a ones row appended
    qa = np.ones((BH, 65, N), dtype=BF_NP)
    qa[:, :64, :] = qf.transpose(0, 2, 1).astype(BF_NP)
    ka = np.ones((BH, 65, N), dtype=BF_NP)
    ka[:, :64, :] = kf.transpose(0, 2, 1).astype(BF_NP)

    rpet = np.zeros((D, 2048), dtype=BF_NP)
    rpet[:, :2047] = np.ascontiguousarray(rpe[::-1].T).astype(BF_NP)

    # vaug: [BH, 128, NB, 65] with ones col; vsum: [BH, 65] = [colsumV | N]
    vaug = np.ones((BH, 128, NB, 65), dtype=BF_NP)
    vaug[:, :, :, :64] = vf.reshape(BH, NB, 128, D).transpose(0, 2, 1, 3
                                                              ).astype(BF_NP)
    vsum = np.empty((BH, 65), dtype=np.float32)
    vsum[:, :64] = vf.sum(axis=1)
    vsum[:, 64] = float(N)
    vsum = vsum.astype(BF_NP)

    if _cached_nc is None:
        _cached_nc = _build()
    nc = _cached_nc

    in_maps = []
    for c in range(NCORES):
        hs = slice(c * HPC, (c + 1) * HPC)
        in_maps.append({
            "qa": qa[hs].transpose(1, 0, 2).reshape(65, HPC * N),
            "ka": ka[hs].transpose(1, 0, 2).reshape(65, HPC * N),
            "rpet": rpet,
            "vaug": vaug[hs].transpose(1, 0, 2, 3).reshape(128,
                                                           HPC * NB * 65),
            "vsum": vsum[hs].reshape(1, HPC * 65),
        })

    res = bass_utils.run_bass_kernel_spmd(
        nc, in_maps, core_ids=list(range(NCORES)), trace=TRACE)
    if TRACE:
        print(f"HW exec time: {res.exec_time_ns} ns")
        if res.instructions_and_trace is not None:
            print("trace:", res.instructions_and_trace[1])

    o = np.concatenate([r["o"] for r in res.results], axis=0)
    return o.reshape(B, H, N, D).astype(np.float32)
